# Initial kernel scaffold
#
"""Mamba-core (4-layer) Trainium2 Bass kernel.

Sharding: data-parallel over batch B=8 across 8 NeuronCores (one sample per
core, zero collectives).  Per core, all activations live in SBUF in
[feature, time] layout:

  - in_proj + causal depthwise conv are fused: conv taps are folded into 4
    time-shifted accumulating matmuls (PSUM accumulation over taps).
  - dt = softplus(...) and SiLU gates run on the scalar (ACT) engine with
    per-partition bias vectors.
  - The selective scan runs as native `tensor_tensor_scan` instructions
    (h = a*h + b along the time axis, fp32 internal state), one [128, 512]
    tile per (d_inner-half, state-n, time-chunk).
  - a = exp(-(n+1)*dt) comes straight from the ACT engine (Exp with
    scale=-(n+1)).
  - b = dtu * B_n and the readout h * C_n need B/C rows broadcast across
    partitions: rows are replicated with K=1 ones-matmuls on the tensor
    engine (PSUM holds the replicated rows).
  - y = sum_n C_n*h_n is accumulated in PSUM via identity matmuls.
"""

import os
import numpy as np

DM = 128        # d_model
DI = 256        # d_inner
NDH = 2         # d_inner halves of 128
NST = 16        # d_state
RNK = 8         # dt_rank
L = 4096
LAYERS = 4
DCONV = 4
CH = 512        # time chunk (one PSUM bank)
NCH = L // CH   # 8
QCH = 2         # chunks per quarter (y-acc PSUM granularity)
NQ = NCH // QCH  # 4 quarters
B = 8
NCORES = 8

F32 = "float32"
BF16 = "bfloat16"

# dtype config (flip these for perf/precision trades)
DT_DT = BF16    # dt tensor
DTU_DT = BF16   # dtu tensor
SZ_DT = F32     # silu(z) tensor
A_DT = F32      # scan decay operand
BT_DT = F32     # scan input operand
H_DT = F32      # scan output
TMP_DT = F32    # readout product


def prep_weights(inputs):
    """Host-side weight preprocessing (numpy, tiny)."""
    in_w = inputs["in_proj_w"]    # [4, 512, 128]
    cw = inputs["conv_w"]         # [4, 256, 4]
    cb = inputs["conv_b"]         # [4, 256]
    xp_w = inputs["x_proj_w"]     # [4, 40, 256]
    dtp_w = inputs["dt_proj_w"]   # [4, 256, 8]
    dtp_b = inputs["dt_proj_b"]   # [4, 256]
    Dp = inputs["D"]              # [4, 256]
    out_w = inputs["out_proj_w"]  # [4, 128, 256]

    wz = np.ascontiguousarray(np.transpose(in_w[:, DI:, :], (0, 2, 1)))  # [4,128,256]
    # conv folded into in_proj: wxa[l, kd, k*DI+m] = cw[l, m, k] * in_w[l, m, kd]
    wxa = np.einsum("lmk,lmd->ldkm", cw, in_w[:, :DI, :])                # [4,128,4,256]
    wxa = np.ascontiguousarray(wxa.reshape(LAYERS, DM, DCONV * DI))
    # wxp[l, ksub, dh*96 + seg]: x_proj output padded to M=96 so the PSUM
    # splits land on 32-aligned partitions: dtraw @ 0:8, Bm @ 32:48, Cm @ 64:80
    wxp_t = np.transpose(xp_w.reshape(LAYERS, 40, NDH, DM), (0, 3, 2, 1))  # [l,ksub,dh,40]
    wxp = np.zeros((LAYERS, DM, NDH, 96), np.float32)
    wxp[:, :, :, 0:RNK] = wxp_t[:, :, :, 0:RNK]
    wxp[:, :, :, 32:32 + NST] = wxp_t[:, :, :, RNK:RNK + NST]
    wxp[:, :, :, 64:64 + NST] = wxp_t[:, :, :, RNK + NST:RNK + 2 * NST]
    wxp = np.ascontiguousarray(wxp.reshape(LAYERS, DM, NDH * 96))
    wdt = np.ascontiguousarray(np.transpose(dtp_w, (0, 2, 1)))           # [4,8,256]
    # wo[l, ksub, dh*128+m] = out_w[l, m, dh*128+ksub]
    wo = np.transpose(out_w.reshape(LAYERS, DM, NDH, DM), (0, 3, 2, 1))
    wo = np.ascontiguousarray(wo.reshape(LAYERS, DM, NDH * DM))
    vecs = np.zeros((LAYERS, DM, 6), np.float32)
    for dh in range(NDH):
        s = slice(dh * DM, (dh + 1) * DM)
        vecs[:, :, 0 + dh] = cb[:, s]
        vecs[:, :, 2 + dh] = dtp_b[:, s]
        vecs[:, :, 4 + dh] = Dp[:, s]
    import ml_dtypes
    # selp[32+k or 64+k, n*128+p] = 1 iff k == n — row-n replicator lhsT,
    # placed at partition bases 32 and 64 so lhsT base matches the rhs base
    # (Bm rows live at pjs[32:48], Cm rows at pjs[64:80]).
    sel = np.zeros((80, NST * DM), np.float32)
    for n in range(NST):
        sel[32 + n, n * DM:(n + 1) * DM] = 1.0
        sel[64 + n, n * DM:(n + 1) * DM] = 1.0
    return {
        "wz": wz.astype(np.float32),
        "wxa": wxa.astype(np.float32),
        "wxp": wxp.astype(np.float32),
        "wdt": wdt.astype(ml_dtypes.bfloat16),
        "wo": wo.astype(np.float32),
        "vecs": vecs,
        "ident": np.eye(DM, dtype=np.float32),
        "sel": sel.astype(ml_dtypes.bfloat16),
    }


def build_program(layers=LAYERS):
    import concourse.bass as bass
    import concourse.tile as tile
    from concourse import bacc, mybir
    from contextlib import ExitStack

    f32 = mybir.dt.float32
    bf16 = mybir.dt.bfloat16
    DT = {F32: f32, BF16: bf16}
    AF = mybir.ActivationFunctionType
    OP = mybir.AluOpType

    nc = bacc.Bacc("TRN2")

    xT = nc.dram_tensor("xT", [DM, L + 3], f32, kind="ExternalInput")
    wz_d = nc.dram_tensor("wz", [LAYERS, DM, DI], f32, kind="ExternalInput")
    wxa_d = nc.dram_tensor("wxa", [LAYERS, DM, DCONV * DI], f32, kind="ExternalInput")
    wxp_d = nc.dram_tensor("wxp", [LAYERS, DM, NDH * 96], f32, kind="ExternalInput")
    wdt_d = nc.dram_tensor("wdt", [LAYERS, RNK, DI], bf16, kind="ExternalInput")
    wo_d = nc.dram_tensor("wo", [LAYERS, DM, NDH * DM], f32, kind="ExternalInput")
    vecs_d = nc.dram_tensor("vecs", [LAYERS, DM, 6], f32, kind="ExternalInput")
    ident_d = nc.dram_tensor("ident", [DM, DM], f32, kind="ExternalInput")
    sel_d = nc.dram_tensor("sel", [80, NST * DM], bf16, kind="ExternalInput")
    out_d = nc.dram_tensor("out", [DM, L], f32, kind="ExternalOutput")

    with tile.TileContext(nc) as tc, ExitStack() as ctx:
        pers = ctx.enter_context(tc.tile_pool(name="pers", bufs=1))
        wts = ctx.enter_context(tc.tile_pool(name="wts", bufs=2))
        work = ctx.enter_context(tc.tile_pool(name="work", bufs=2))
        ps = ctx.enter_context(tc.tile_pool(name="ps", bufs=4, space="PSUM"))
        psacc = ctx.enter_context(tc.tile_pool(name="psacc", bufs=1, space="PSUM"))

        xt = pers.tile([DM, L + 3], f32, tag="xt", name="xt")
        nc.sync.dma_start(xt[:], xT[:])
        ident = pers.tile([DM, DM], f32, tag="ident", name="ident")
        nc.sync.dma_start(ident[:], ident_d[:])
        sel = pers.tile([80, NST * DM], bf16, tag="sel", name="sel")
        nc.sync.dma_start(sel[:], sel_d[:])

        xa = [pers.tile([DM, L], f32, tag=f"xa{dh}", name=f"xa{dh}") for dh in range(NDH)]
        dts = [pers.tile([DM, L], DT[DT_DT], tag=f"dt{dh}", name=f"dt{dh}") for dh in range(NDH)]
        dtu = [pers.tile([DM, L], DT[DTU_DT], tag=f"dtu{dh}", name=f"dtu{dh}") for dh in range(NDH)]
        sz = [pers.tile([DM, L], DT[SZ_DT], tag=f"sz{dh}", name=f"sz{dh}") for dh in range(NDH)]
        # pjs holds the x_proj outputs: dtraw @ rows 0:8, Bm @ 32:48, Cm @ 64:80
        pjs = pers.tile([96, L], bf16, tag="pjs", name="pjs")
        hlast = pers.tile([DM, NDH * NST], f32, tag="hlast", name="hlast")

        for layer in range(layers):
            wl = layer % LAYERS
            # ---- per-layer weights -> SBUF (double-buffered pool) ----
            w_z = wts.tile([DM, DI], f32, tag="w_z", name="w_z")
            nc.sync.dma_start(w_z[:], wz_d[wl])
            w_xa = wts.tile([DM, DCONV * DI], f32, tag="w_xa", name="w_xa")
            nc.sync.dma_start(w_xa[:], wxa_d[wl])
            w_xp = wts.tile([DM, NDH * 96], f32, tag="w_xp", name="w_xp")
            nc.sync.dma_start(w_xp[:], wxp_d[wl])
            w_dt = wts.tile([RNK, DI], bf16, tag="w_dt", name="w_dt")
            nc.sync.dma_start(w_dt[:], wdt_d[wl])
            w_o = wts.tile([DM, NDH * DM], f32, tag="w_o", name="w_o")
            nc.sync.dma_start(w_o[:], wo_d[wl])
            vec = wts.tile([DM, 6], f32, tag="vec", name="vec")
            nc.sync.dma_start(vec[:], vecs_d[wl])

            # ---- stage A1: in_proj+conv, silu gates (Sigmoid table), x_proj ----
            for cc in range(NCH):
                t0 = cc * CH
                for dh in range(NDH):
                    mslc = slice(dh * DM, (dh + 1) * DM)
                    # z path: silu(z) = z * sigmoid(z)
                    p_z = ps.tile([DM, CH], f32, tag="rep", name="rep")
                    nc.tensor.matmul(p_z[:], w_z[:, mslc], xt[:, t0 + 3:t0 + 3 + CH],
                                     start=True, stop=True)
                    sg = work.tile([DM, CH], f32, tag="sg", name="sg")
                    nc.scalar.activation(sg[:], p_z[:], AF.Sigmoid)
                    nc.vector.tensor_tensor(sz[dh][:, t0:t0 + CH], p_z[:], sg[:],
                                            OP.mult)
                    # xa path: conv folded as 4 shifted accumulating matmuls
                    p_xa = ps.tile([DM, CH], f32, tag="rep", name="rep")
                    for k in range(DCONV):
                        nc.tensor.matmul(
                            p_xa[:], w_xa[:, k * DI + dh * DM:k * DI + (dh + 1) * DM],
                            xt[:, t0 + k:t0 + k + CH],
                            start=(k == 0), stop=(k == DCONV - 1))
                    ux = work.tile([DM, CH], f32, tag="ux", name="ux")
                    nc.scalar.activation(ux[:], p_xa[:], AF.Identity,
                                         bias=vec[:, 0 + dh:1 + dh])
                    sgx = work.tile([DM, CH], f32, tag="sg", name="sg")
                    nc.scalar.activation(sgx[:], p_xa[:], AF.Sigmoid,
                                         bias=vec[:, 0 + dh:1 + dh])
                    nc.vector.tensor_tensor(xa[dh][:, t0:t0 + CH], ux[:], sgx[:],
                                            OP.mult)
                # x_proj: [96, CH] -> split to dtraw/Bt/Ct (32-aligned PSUM reads)
                p_pj = ps.tile([96, CH], f32, tag="rep", name="rep")
                for dh in range(NDH):
                    nc.tensor.matmul(p_pj[:], w_xp[:, dh * 96:(dh + 1) * 96],
                                     xa[dh][:, t0:t0 + CH],
                                     start=(dh == 0), stop=(dh == NDH - 1))
                nc.scalar.copy(pjs[:, t0:t0 + CH], p_pj[:])
            # ---- stage A2: dt = softplus(...) via Exp+Ln, batched per function
            # so the ACT table set is loaded once per pass (Exp and Ln live in
            # different PWP table sets here).
            for cc in range(NCH):
                t0 = cc * CH
                for dh in range(NDH):
                    mslc = slice(dh * DM, (dh + 1) * DM)
                    p_dt = ps.tile([DM, CH], f32, tag="rep", name="rep")
                    nc.tensor.matmul(p_dt[:], w_dt[:, mslc], pjs[0:RNK, t0:t0 + CH],
                                     start=True, stop=True)
                    # dts <- exp(dt_raw@W + b), overwritten by Ln below
                    nc.scalar.activation(dts[dh][:, t0:t0 + CH], p_dt[:], AF.Exp,
                                         bias=vec[:, 2 + dh:3 + dh])
            for cc in range(NCH):
                t0 = cc * CH
                for dh in range(NDH):
                    nc.scalar.activation(dts[dh][:, t0:t0 + CH],
                                         dts[dh][:, t0:t0 + CH], AF.Ln, bias=1.0)
                    nc.vector.tensor_tensor(dtu[dh][:, t0:t0 + CH],
                                            dts[dh][:, t0:t0 + CH],
                                            xa[dh][:, t0:t0 + CH], OP.mult)

            # ---- stage B + C: scan per quarter ----
            for q in range(NQ):
                q0 = q * QCH * CH
                acc = [psacc.tile([DM, QCH * CH], f32, tag=f"acc{dh}", name=f"acc{dh}") for dh in range(NDH)]
                hprev = [[None] * NST for _ in range(NDH)]
                for n in range(NST):
                    for c in range(QCH):
                        t0 = q0 + c * CH
                        # replicate B_n, C_n rows across 128 partitions
                        # (K=16 selector matmul; operand bases stay at 0)
                        brep = ps.tile([DM, CH], f32, tag="rep", name="rep")
                        nc.tensor.matmul(brep[:], sel[32:32 + NST, n * DM:(n + 1) * DM],
                                         pjs[32:32 + NST, t0:t0 + CH],
                                         start=True, stop=True)
                        crep = ps.tile([DM, CH], f32, tag="rep", name="rep")
                        nc.tensor.matmul(crep[:], sel[64:64 + NST, n * DM:(n + 1) * DM],
                                         pjs[64:64 + NST, t0:t0 + CH],
                                         start=True, stop=True)
                        for dh in range(NDH):
                            at = work.tile([DM, CH], DT[A_DT], tag="a", name="a")
                            nc.scalar.activation(at[:], dts[dh][:, t0:t0 + CH], AF.Exp,
                                                 scale=-float(n + 1))
                            bt = work.tile([DM, CH], DT[BT_DT], tag="b", name="b")
                            nc.vector.tensor_tensor(bt[:], dtu[dh][:, t0:t0 + CH],
                                                    brep[:], OP.mult)
                            ht = work.tile([DM, CH], DT[H_DT], tag=f"h{dh}",
                                           name=f"h{dh}")
                            if c == 0:
                                init = hlast[:, dh * NST + n:dh * NST + n + 1] \
                                    if (q > 0) else 0.0
                            else:
                                init = hprev[dh][n][:, CH - 1:CH]
                            nc.vector.tensor_tensor_scan(ht[:], at[:], bt[:], init,
                                                         OP.mult, OP.add)
                            hprev[dh][n] = ht
                            tmp = work.tile([DM, CH], DT[TMP_DT], tag="tmp", name="tmp")
                            nc.vector.tensor_tensor(tmp[:], ht[:], crep[:], OP.mult)
                            nc.tensor.matmul(acc[dh][:, c * CH:(c + 1) * CH],
                                             ident[:], tmp[:],
                                             start=(n == 0), stop=(n == NST - 1))
                    for dh in range(NDH):
                        if q < NQ - 1:
                            nc.vector.tensor_copy(
                                hlast[:, dh * NST + n:dh * NST + n + 1],
                                hprev[dh][n][:, CH - 1:CH])
                # stage C for this quarter
                for c in range(QCH):
                    t0 = q0 + c * CH
                    ygs = []
                    for dh in range(NDH):
                        y2 = work.tile([DM, CH], f32, tag="y2", name="y2")
                        nc.vector.scalar_tensor_tensor(
                            y2[:], xa[dh][:, t0:t0 + CH], vec[:, 4 + dh:5 + dh],
                            acc[dh][:, c * CH:(c + 1) * CH], OP.mult, OP.add)
                        yg = work.tile([DM, CH], f32, tag="yg", name="yg")
                        nc.vector.tensor_tensor(yg[:], y2[:], sz[dh][:, t0:t0 + CH],
                                                OP.mult)
                        ygs.append(yg)
                    p_x = ps.tile([DM, CH], f32, tag="rep", name="rep")
                    for dh in range(NDH):
                        nc.tensor.matmul(p_x[:], w_o[:, dh * DM:(dh + 1) * DM],
                                         ygs[dh][:], start=(dh == 0), stop=(dh == NDH - 1))
                    if layer < layers - 1:
                        nc.scalar.copy(xt[:, t0 + 3:t0 + 3 + CH], p_x[:])
                    else:
                        ot = work.tile([DM, CH], f32, tag="ot", name="ot")
                        nc.scalar.copy(ot[:], p_x[:])
                        nc.sync.dma_start(out_d[:, t0:t0 + CH], ot[:])
    nc.compile()
    return nc


def numpy_sim(inputs):
    """Tile-level numpy simulation of the exact device algorithm."""
    w = prep_weights(inputs)
    x = inputs["x"]  # [B, L, DM]
    out = np.empty((B, L, DM), np.float32)

    def q(v, dt):
        if dt == BF16:
            import ml_dtypes
            return v.astype(ml_dtypes.bfloat16).astype(np.float32)
        return v.astype(np.float32)

    def silu(v):
        return v / (1 + np.exp(-v))

    for bb in range(B):
        xt = np.zeros((DM, L + 3), np.float32)
        xt[:, 3:] = x[bb].T
        for layer in range(LAYERS):
            vec = w["vecs"][layer]
            xa, dts, dtu_, sz_ = [], [], [], []
            for dh in range(NDH):
                mslc = slice(dh * DM, (dh + 1) * DM)
                zp = w["wz"][layer][:, mslc].T @ xt[:, 3:]
                sz_.append(q(silu(zp), SZ_DT))
                pxa = np.zeros((DM, L), np.float32)
                for k in range(DCONV):
                    pxa += w["wxa"][layer][:, k * DI + dh * DM:k * DI + (dh + 1) * DM].T \
                        @ xt[:, k:k + L]
                xa.append(silu(pxa + vec[:, 0 + dh:1 + dh]))
            proj = np.zeros((96, L), np.float32)
            for dh in range(NDH):
                proj += w["wxp"][layer][:, dh * 96:(dh + 1) * 96].T @ xa[dh]
            dtraw = q(proj[0:RNK], BF16)
            Btl = q(proj[32:32 + NST], BF16)
            Ctl = q(proj[64:64 + NST], BF16)
            wdt_f = np.asarray(w["wdt"][layer], np.float32)
            for dh in range(NDH):
                mslc = slice(dh * DM, (dh + 1) * DM)
                pdt = wdt_f[:, mslc].T @ dtraw
                e = q(np.exp(pdt + vec[:, 2 + dh:3 + dh]), DT_DT)
                dts.append(q(np.log1p(e), DT_DT))
                dtu_.append(q(dts[dh] * xa[dh], DTU_DT))
            ys = []
            for dh in range(NDH):
                acc = np.zeros((DM, L), np.float32)
                for n in range(NST):
                    a = q(np.exp(-(n + 1) * dts[dh]), A_DT)
                    bt = q(dtu_[dh] * Btl[n:n + 1], BT_DT)
                    h = np.zeros((DM, L), np.float32)
                    s = np.zeros(DM, np.float32)
                    for t in range(L):
                        s = a[:, t] * s + bt[:, t]
                        h[:, t] = s
                    h = q(h, H_DT)
                    acc += q(h * Ctl[n:n + 1], TMP_DT)
                y2 = xa[dh] * vec[:, 4 + dh:5 + dh] + acc
                ys.append(y2 * sz_[dh])
            px = np.zeros((DM, L), np.float32)
            for dh in range(NDH):
                px += w["wo"][layer][:, dh * DM:(dh + 1) * DM].T @ ys[dh]
            xt[:, 3:] = px
        out[bb] = xt[:, 3:].T
    return out


_last_results = None


def kernel(**inputs):
    global _last_results
    from concourse.bass_utils import run_bass_kernel_spmd

    w = prep_weights(inputs)
    x = inputs["x"]
    nc = build_program()
    in_maps = []
    for bb in range(NCORES):
        xt = np.zeros((DM, L + 3), np.float32)
        xt[:, 3:] = x[bb].T
        m = {"xT": xt}
        m.update(w)
        in_maps.append(m)
    # the axon NTFF hook is absent in this container; never trace here
    os.environ["BASS_NEVER_TRACE"] = "1"
    br = run_bass_kernel_spmd(nc, in_maps, core_ids=list(range(NCORES)),
                              trace=False)
    _last_results = br
    out = np.empty((B, L, DM), np.float32)
    for bb in range(NCORES):
        out[bb] = br.results[bb]["out"].T
    return out



# revision 1
# speedup vs baseline: 1.0165x; 1.0165x over previous
"""Mamba-core (4-layer) Trainium2 Bass kernel.

Sharding: data-parallel over batch B=8 across 8 NeuronCores (one sample per
core, zero collectives).  Per core, all activations live in SBUF in
[feature, time] layout:

  - in_proj + causal depthwise conv are fused: conv taps are folded into 4
    time-shifted accumulating matmuls (PSUM accumulation over taps).
  - dt = softplus(...) and SiLU gates run on the scalar (ACT) engine with
    per-partition bias vectors.
  - The selective scan runs as native `tensor_tensor_scan` instructions
    (h = a*h + b along the time axis, fp32 internal state), one [128, 512]
    tile per (d_inner-half, state-n, time-chunk).
  - a = exp(-(n+1)*dt) comes straight from the ACT engine (Exp with
    scale=-(n+1)).
  - b = dtu * B_n and the readout h * C_n need B/C rows broadcast across
    partitions: rows are replicated with K=1 ones-matmuls on the tensor
    engine (PSUM holds the replicated rows).
  - y = sum_n C_n*h_n is accumulated in PSUM via identity matmuls.
"""

import os
import numpy as np

DM = 128        # d_model
DI = 256        # d_inner
NDH = 2         # d_inner halves of 128
NST = 16        # d_state
RNK = 8         # dt_rank
L = 4096
LAYERS = 4
DCONV = 4
CH = 512        # time chunk (one PSUM bank)
NCH = L // CH   # 8
QCH = 2         # chunks per quarter (y-acc PSUM granularity)
NQ = NCH // QCH  # 4 quarters
B = 8
NCORES = 8

F32 = "float32"
BF16 = "bfloat16"

# dtype config (flip these for perf/precision trades)
DT_DT = BF16    # dt tensor
DTU_DT = BF16   # dtu tensor
SZ_DT = F32     # silu(z) tensor
A_DT = F32      # scan decay operand
BT_DT = F32     # scan input operand
H_DT = F32      # scan output
TMP_DT = F32    # readout product


def prep_weights(inputs):
    """Host-side weight preprocessing (numpy, tiny)."""
    in_w = inputs["in_proj_w"]    # [4, 512, 128]
    cw = inputs["conv_w"]         # [4, 256, 4]
    cb = inputs["conv_b"]         # [4, 256]
    xp_w = inputs["x_proj_w"]     # [4, 40, 256]
    dtp_w = inputs["dt_proj_w"]   # [4, 256, 8]
    dtp_b = inputs["dt_proj_b"]   # [4, 256]
    Dp = inputs["D"]              # [4, 256]
    out_w = inputs["out_proj_w"]  # [4, 128, 256]

    wz = np.ascontiguousarray(np.transpose(in_w[:, DI:, :], (0, 2, 1)))  # [4,128,256]
    # conv folded into in_proj: wxa[l, kd, k*DI+m] = cw[l, m, k] * in_w[l, m, kd]
    wxa = np.einsum("lmk,lmd->ldkm", cw, in_w[:, :DI, :])                # [4,128,4,256]
    wxa = np.ascontiguousarray(wxa.reshape(LAYERS, DM, DCONV * DI))
    # wxp[l, ksub, dh*96 + seg]: x_proj output padded to M=96 so the PSUM
    # splits land on 32-aligned partitions: dtraw @ 0:8, Bm @ 32:48, Cm @ 64:80
    wxp_t = np.transpose(xp_w.reshape(LAYERS, 40, NDH, DM), (0, 3, 2, 1))  # [l,ksub,dh,40]
    wxp = np.zeros((LAYERS, DM, NDH, 96), np.float32)
    wxp[:, :, :, 0:RNK] = wxp_t[:, :, :, 0:RNK]
    wxp[:, :, :, 32:32 + NST] = wxp_t[:, :, :, RNK:RNK + NST]
    wxp[:, :, :, 64:64 + NST] = wxp_t[:, :, :, RNK + NST:RNK + 2 * NST]
    wxp = np.ascontiguousarray(wxp.reshape(LAYERS, DM, NDH * 96))
    wdt = np.ascontiguousarray(np.transpose(dtp_w, (0, 2, 1)))           # [4,8,256]
    # wo[l, ksub, dh*128+m] = out_w[l, m, dh*128+ksub]
    wo = np.transpose(out_w.reshape(LAYERS, DM, NDH, DM), (0, 3, 2, 1))
    wo = np.ascontiguousarray(wo.reshape(LAYERS, DM, NDH * DM))
    vecs = np.zeros((LAYERS, DM, 6), np.float32)
    for dh in range(NDH):
        s = slice(dh * DM, (dh + 1) * DM)
        vecs[:, :, 0 + dh] = cb[:, s]
        vecs[:, :, 2 + dh] = dtp_b[:, s]
        vecs[:, :, 4 + dh] = Dp[:, s]
    import ml_dtypes
    # selp[32+k or 64+k, n*128+p] = 1 iff k == n — row-n replicator lhsT,
    # placed at partition bases 32 and 64 so lhsT base matches the rhs base
    # (Bm rows live at pjs[32:48], Cm rows at pjs[64:80]).
    sel = np.zeros((80, NST * DM), np.float32)
    for n in range(NST):
        sel[32 + n, n * DM:(n + 1) * DM] = 1.0
        sel[64 + n, n * DM:(n + 1) * DM] = 1.0
    return {
        "wz": wz.astype(np.float32),
        "wxa": wxa.astype(np.float32),
        "wxp": wxp.astype(np.float32),
        "wdt": wdt.astype(ml_dtypes.bfloat16),
        "wo": wo.astype(np.float32),
        "vecs": vecs,
        "ident": np.eye(DM, dtype=np.float32),
        "sel": sel.astype(ml_dtypes.bfloat16),
    }


def build_program(layers=LAYERS):
    import concourse.bass as bass
    import concourse.tile as tile
    from concourse import bacc, mybir
    from contextlib import ExitStack

    f32 = mybir.dt.float32
    bf16 = mybir.dt.bfloat16
    DT = {F32: f32, BF16: bf16}
    AF = mybir.ActivationFunctionType
    OP = mybir.AluOpType

    nc = bacc.Bacc("TRN2")

    xT = nc.dram_tensor("xT", [DM, L + 3], f32, kind="ExternalInput")
    wz_d = nc.dram_tensor("wz", [LAYERS, DM, DI], f32, kind="ExternalInput")
    wxa_d = nc.dram_tensor("wxa", [LAYERS, DM, DCONV * DI], f32, kind="ExternalInput")
    wxp_d = nc.dram_tensor("wxp", [LAYERS, DM, NDH * 96], f32, kind="ExternalInput")
    wdt_d = nc.dram_tensor("wdt", [LAYERS, RNK, DI], bf16, kind="ExternalInput")
    wo_d = nc.dram_tensor("wo", [LAYERS, DM, NDH * DM], f32, kind="ExternalInput")
    vecs_d = nc.dram_tensor("vecs", [LAYERS, DM, 6], f32, kind="ExternalInput")
    ident_d = nc.dram_tensor("ident", [DM, DM], f32, kind="ExternalInput")
    sel_d = nc.dram_tensor("sel", [80, NST * DM], bf16, kind="ExternalInput")
    out_d = nc.dram_tensor("out", [DM, L], f32, kind="ExternalOutput")

    with tile.TileContext(nc) as tc, ExitStack() as ctx:
        pers = ctx.enter_context(tc.tile_pool(name="pers", bufs=1))
        wts = ctx.enter_context(tc.tile_pool(name="wts", bufs=2))
        work = ctx.enter_context(tc.tile_pool(name="work", bufs=2))
        ps = ctx.enter_context(tc.tile_pool(name="ps", bufs=4, space="PSUM"))
        psacc = ctx.enter_context(tc.tile_pool(name="psacc", bufs=1, space="PSUM"))

        xt = pers.tile([DM, L + 3], f32, tag="xt", name="xt")
        nc.sync.dma_start(xt[:], xT[:])
        ident = pers.tile([DM, DM], f32, tag="ident", name="ident")
        nc.sync.dma_start(ident[:], ident_d[:])
        sel = pers.tile([80, NST * DM], bf16, tag="sel", name="sel")
        nc.sync.dma_start(sel[:], sel_d[:])

        xa = [pers.tile([DM, L], f32, tag=f"xa{dh}", name=f"xa{dh}") for dh in range(NDH)]
        dts = [pers.tile([DM, L], DT[DT_DT], tag=f"dt{dh}", name=f"dt{dh}") for dh in range(NDH)]
        dtu = [pers.tile([DM, L], DT[DTU_DT], tag=f"dtu{dh}", name=f"dtu{dh}") for dh in range(NDH)]
        sz = [pers.tile([DM, L], DT[SZ_DT], tag=f"sz{dh}", name=f"sz{dh}") for dh in range(NDH)]
        # pjs holds the x_proj outputs: dtraw @ rows 0:8, Bm @ 32:48, Cm @ 64:80
        pjs = pers.tile([96, L], bf16, tag="pjs", name="pjs")
        hlast = pers.tile([DM, NDH * NST], f32, tag="hlast", name="hlast")

        for layer in range(layers):
            wl = layer % LAYERS
            # ---- per-layer weights -> SBUF (double-buffered pool) ----
            w_z = wts.tile([DM, DI], f32, tag="w_z", name="w_z")
            nc.sync.dma_start(w_z[:], wz_d[wl])
            w_xa = wts.tile([DM, DCONV * DI], f32, tag="w_xa", name="w_xa")
            nc.sync.dma_start(w_xa[:], wxa_d[wl])
            w_xp = wts.tile([DM, NDH * 96], f32, tag="w_xp", name="w_xp")
            nc.sync.dma_start(w_xp[:], wxp_d[wl])
            w_dt = wts.tile([RNK, DI], bf16, tag="w_dt", name="w_dt")
            nc.sync.dma_start(w_dt[:], wdt_d[wl])
            w_o = wts.tile([DM, NDH * DM], f32, tag="w_o", name="w_o")
            nc.sync.dma_start(w_o[:], wo_d[wl])
            vec = wts.tile([DM, 6], f32, tag="vec", name="vec")
            nc.sync.dma_start(vec[:], vecs_d[wl])

            # ---- stage A1: in_proj+conv, silu gates (Sigmoid table), x_proj ----
            for cc in range(NCH):
                t0 = cc * CH
                for dh in range(NDH):
                    mslc = slice(dh * DM, (dh + 1) * DM)
                    # z path: silu(z) = z * sigmoid(z)
                    p_z = ps.tile([DM, CH], f32, tag="rep", name="rep")
                    nc.tensor.matmul(p_z[:], w_z[:, mslc], xt[:, t0 + 3:t0 + 3 + CH],
                                     start=True, stop=True)
                    sg = work.tile([DM, CH], f32, tag="sg", name="sg")
                    nc.scalar.activation(sg[:], p_z[:], AF.Sigmoid)
                    nc.vector.tensor_tensor(sz[dh][:, t0:t0 + CH], p_z[:], sg[:],
                                            OP.mult)
                    # xa path: conv folded as 4 shifted accumulating matmuls
                    p_xa = ps.tile([DM, CH], f32, tag="rep", name="rep")
                    for k in range(DCONV):
                        nc.tensor.matmul(
                            p_xa[:], w_xa[:, k * DI + dh * DM:k * DI + (dh + 1) * DM],
                            xt[:, t0 + k:t0 + k + CH],
                            start=(k == 0), stop=(k == DCONV - 1))
                    ux = work.tile([DM, CH], f32, tag="ux", name="ux")
                    nc.scalar.activation(ux[:], p_xa[:], AF.Identity,
                                         bias=vec[:, 0 + dh:1 + dh])
                    sgx = work.tile([DM, CH], f32, tag="sg", name="sg")
                    nc.scalar.activation(sgx[:], p_xa[:], AF.Sigmoid,
                                         bias=vec[:, 0 + dh:1 + dh])
                    nc.vector.tensor_tensor(xa[dh][:, t0:t0 + CH], ux[:], sgx[:],
                                            OP.mult)
                # x_proj: [96, CH] -> split to dtraw/Bt/Ct (32-aligned PSUM reads)
                p_pj = ps.tile([96, CH], f32, tag="rep", name="rep")
                for dh in range(NDH):
                    nc.tensor.matmul(p_pj[:], w_xp[:, dh * 96:(dh + 1) * 96],
                                     xa[dh][:, t0:t0 + CH],
                                     start=(dh == 0), stop=(dh == NDH - 1))
                nc.scalar.copy(pjs[:, t0:t0 + CH], p_pj[:])
            # ---- stage A2: dt = softplus(...) via Exp+Ln, batched per function
            # so the ACT table set is loaded once per pass (Exp and Ln live in
            # different PWP table sets here).
            for cc in range(NCH):
                t0 = cc * CH
                for dh in range(NDH):
                    mslc = slice(dh * DM, (dh + 1) * DM)
                    p_dt = ps.tile([DM, CH], f32, tag="rep", name="rep")
                    nc.tensor.matmul(p_dt[:], w_dt[:, mslc], pjs[0:RNK, t0:t0 + CH],
                                     start=True, stop=True)
                    # dts <- exp(dt_raw@W + b), overwritten by Ln below
                    nc.scalar.activation(dts[dh][:, t0:t0 + CH], p_dt[:], AF.Exp,
                                         bias=vec[:, 2 + dh:3 + dh])
            for cc in range(NCH):
                t0 = cc * CH
                for dh in range(NDH):
                    nc.scalar.activation(dts[dh][:, t0:t0 + CH],
                                         dts[dh][:, t0:t0 + CH], AF.Ln, bias=1.0)
                    nc.vector.tensor_tensor(dtu[dh][:, t0:t0 + CH],
                                            dts[dh][:, t0:t0 + CH],
                                            xa[dh][:, t0:t0 + CH], OP.mult)

            # ---- stage B + C: scan per quarter ----
            for q in range(NQ):
                q0 = q * QCH * CH
                acc = [psacc.tile([DM, QCH * CH], f32, tag=f"acc{dh}", name=f"acc{dh}") for dh in range(NDH)]
                hprev = [[None] * NST for _ in range(NDH)]
                for n in range(NST):
                    for c in range(QCH):
                        t0 = q0 + c * CH
                        # replicate B_n, C_n rows across 128 partitions
                        # (K=16 selector matmul; operand bases stay at 0)
                        brep = ps.tile([DM, CH], f32, tag="rep", name="rep")
                        nc.tensor.matmul(brep[:], sel[32:32 + NST, n * DM:(n + 1) * DM],
                                         pjs[32:32 + NST, t0:t0 + CH],
                                         start=True, stop=True)
                        crep = ps.tile([DM, CH], f32, tag="rep", name="rep")
                        nc.tensor.matmul(crep[:], sel[64:64 + NST, n * DM:(n + 1) * DM],
                                         pjs[64:64 + NST, t0:t0 + CH],
                                         start=True, stop=True)
                        for dh in range(NDH):
                            at = work.tile([DM, CH], DT[A_DT], tag="a", name="a")
                            nc.scalar.activation(at[:], dts[dh][:, t0:t0 + CH], AF.Exp,
                                                 scale=-float(n + 1))
                            bt = work.tile([DM, CH], DT[BT_DT], tag="b", name="b")
                            nc.vector.tensor_tensor(bt[:], dtu[dh][:, t0:t0 + CH],
                                                    brep[:], OP.mult)
                            ht = work.tile([DM, CH], DT[H_DT], tag=f"h{dh}",
                                           name=f"h{dh}")
                            if c == 0:
                                init = hlast[:, dh * NST + n:dh * NST + n + 1] \
                                    if (q > 0) else 0.0
                            else:
                                init = hprev[dh][n][:, CH - 1:CH]
                            nc.vector.tensor_tensor_scan(ht[:], at[:], bt[:], init,
                                                         OP.mult, OP.add)
                            hprev[dh][n] = ht
                            tmp = work.tile([DM, CH], DT[TMP_DT], tag="tmp", name="tmp")
                            nc.vector.tensor_tensor(tmp[:], ht[:], crep[:], OP.mult)
                            nc.tensor.matmul(acc[dh][:, c * CH:(c + 1) * CH],
                                             ident[:], tmp[:],
                                             start=(n == 0), stop=(n == NST - 1))
                    for dh in range(NDH):
                        if q < NQ - 1:
                            nc.vector.tensor_copy(
                                hlast[:, dh * NST + n:dh * NST + n + 1],
                                hprev[dh][n][:, CH - 1:CH])
                # stage C for this quarter
                for c in range(QCH):
                    t0 = q0 + c * CH
                    ygs = []
                    for dh in range(NDH):
                        y2 = work.tile([DM, CH], f32, tag="y2", name="y2")
                        nc.vector.scalar_tensor_tensor(
                            y2[:], xa[dh][:, t0:t0 + CH], vec[:, 4 + dh:5 + dh],
                            acc[dh][:, c * CH:(c + 1) * CH], OP.mult, OP.add)
                        yg = work.tile([DM, CH], f32, tag="yg", name="yg")
                        nc.vector.tensor_tensor(yg[:], y2[:], sz[dh][:, t0:t0 + CH],
                                                OP.mult)
                        ygs.append(yg)
                    p_x = ps.tile([DM, CH], f32, tag="rep", name="rep")
                    for dh in range(NDH):
                        nc.tensor.matmul(p_x[:], w_o[:, dh * DM:(dh + 1) * DM],
                                         ygs[dh][:], start=(dh == 0), stop=(dh == NDH - 1))
                    if layer < layers - 1:
                        nc.scalar.copy(xt[:, t0 + 3:t0 + 3 + CH], p_x[:])
                    else:
                        ot = work.tile([DM, CH], f32, tag="ot", name="ot")
                        nc.scalar.copy(ot[:], p_x[:])
                        nc.sync.dma_start(out_d[:, t0:t0 + CH], ot[:])
    nc.compile()
    return nc


def numpy_sim(inputs):
    """Tile-level numpy simulation of the exact device algorithm."""
    w = prep_weights(inputs)
    x = inputs["x"]  # [B, L, DM]
    out = np.empty((B, L, DM), np.float32)

    def q(v, dt):
        if dt == BF16:
            import ml_dtypes
            return v.astype(ml_dtypes.bfloat16).astype(np.float32)
        return v.astype(np.float32)

    def silu(v):
        return v / (1 + np.exp(-v))

    for bb in range(B):
        xt = np.zeros((DM, L + 3), np.float32)
        xt[:, 3:] = x[bb].T
        for layer in range(LAYERS):
            vec = w["vecs"][layer]
            xa, dts, dtu_, sz_ = [], [], [], []
            for dh in range(NDH):
                mslc = slice(dh * DM, (dh + 1) * DM)
                zp = w["wz"][layer][:, mslc].T @ xt[:, 3:]
                sz_.append(q(silu(zp), SZ_DT))
                pxa = np.zeros((DM, L), np.float32)
                for k in range(DCONV):
                    pxa += w["wxa"][layer][:, k * DI + dh * DM:k * DI + (dh + 1) * DM].T \
                        @ xt[:, k:k + L]
                xa.append(silu(pxa + vec[:, 0 + dh:1 + dh]))
            proj = np.zeros((96, L), np.float32)
            for dh in range(NDH):
                proj += w["wxp"][layer][:, dh * 96:(dh + 1) * 96].T @ xa[dh]
            dtraw = q(proj[0:RNK], BF16)
            Btl = q(proj[32:32 + NST], BF16)
            Ctl = q(proj[64:64 + NST], BF16)
            wdt_f = np.asarray(w["wdt"][layer], np.float32)
            for dh in range(NDH):
                mslc = slice(dh * DM, (dh + 1) * DM)
                pdt = wdt_f[:, mslc].T @ dtraw
                e = q(np.exp(pdt + vec[:, 2 + dh:3 + dh]), DT_DT)
                dts.append(q(np.log1p(e), DT_DT))
                dtu_.append(q(dts[dh] * xa[dh], DTU_DT))
            ys = []
            for dh in range(NDH):
                acc = np.zeros((DM, L), np.float32)
                for n in range(NST):
                    a = q(np.exp(-(n + 1) * dts[dh]), A_DT)
                    bt = q(dtu_[dh] * Btl[n:n + 1], BT_DT)
                    h = np.zeros((DM, L), np.float32)
                    s = np.zeros(DM, np.float32)
                    for t in range(L):
                        s = a[:, t] * s + bt[:, t]
                        h[:, t] = s
                    h = q(h, H_DT)
                    acc += q(h * Ctl[n:n + 1], TMP_DT)
                y2 = xa[dh] * vec[:, 4 + dh:5 + dh] + acc
                ys.append(y2 * sz_[dh])
            px = np.zeros((DM, L), np.float32)
            for dh in range(NDH):
                px += w["wo"][layer][:, dh * DM:(dh + 1) * DM].T @ ys[dh]
            xt[:, 3:] = px
        out[bb] = xt[:, 3:].T
    return out


_last_results = None


def kernel(**inputs):
    global _last_results
    from concourse.bass_utils import run_bass_kernel_spmd

    w = prep_weights(inputs)
    x = inputs["x"]
    nc = build_program()
    in_maps = []
    for bb in range(NCORES):
        xt = np.zeros((DM, L + 3), np.float32)
        xt[:, 3:] = x[bb].T
        m = {"xT": xt}
        m.update(w)
        in_maps.append(m)
    # the axon NTFF hook is absent in this container; never trace here
    os.environ["BASS_NEVER_TRACE"] = "1"
    br = run_bass_kernel_spmd(nc, in_maps, core_ids=list(range(NCORES)),
                              trace=False)
    _last_results = br
    out = np.empty((B, L, DM), np.float32)
    for bb in range(NCORES):
        out[bb] = br.results[bb]["out"].T
    return out

